# revision 1
# baseline (speedup 1.0000x reference)
"""BitNet attention SPMD kernel for 8 Trainium2 NeuronCores.

Problem: nn_BitNetAttention (B=2, N=2048, C=768, H=12, D=64).

Sharding: data-parallel over batch (2 groups of 4 cores); within a group each
core owns 512 query tokens.  Each core recomputes the full K/V for its batch
(collective-free), runs its N/4 x N attention slab for all 12 heads, and
produces its own [512, 768] slice of the final output.  The host concatenates
the 8 slices.

Numerics:
- BitNet quantized matmuls (qkv, proj) run as exact integer arithmetic: int8
  activations and ternary weights are exactly representable in bf16, and fp32
  PSUM accumulation of <=2^24 magnitudes is exact.  Dequantization scales are
  folded into cheap per-token column/broadcast multiplies.
- Attention matmuls (QK^T, AV) run in float32r (full PE rate, ~1.6e-4 rel).
- Softmax skips the max-subtraction (logits are O(1) by construction); the
  denominator comes from a ones-column appended to V and is divided out after
  a small per-head PE transpose.
"""
import sys
sys.path.insert(0, "/opt/trn_rl_repo")

import numpy as np
from contextlib import ExitStack

import concourse.bass as bass
import concourse.mybir as mybir
import concourse.tile as tile
import concourse.bacc as bacc
from concourse.bass_utils import run_bass_kernel_spmd

dt = mybir.dt
AF = mybir.ActivationFunctionType
ALU = mybir.AluOpType
AX = mybir.AxisListType

B, N, C = 2, 2048, 768
H, D = 12, 64
NQ = N // 4              # 512 query tokens per core
TKV = N // 128           # 16 kv token chunks
TQ = NQ // 128           # 4 q token chunks
NCC = C // 128           # 6 contraction chunks
EPS = 1e-5
MAGIC = 12582912.0       # 1.5*2^23: x+MAGIC lands in [2^23,2^24) where ulp=1

_CACHE = {}


QUAKE = 0x5F3759DF


def _rsqrt_col(nc, st, ms):
    """rstd = 1/sqrt(ms) on DVE via the bit-trick seed + 2 Newton steps.

    Error after two steps is ~3e-11 rel — indistinguishable from a rounded
    fp32 rsqrt.  Avoids the ACT Sqrt table set (keeps ACT exp-only, so the
    program pays a single act-table load).
    """
    ihalf = st.tile([128, 1], dt.int32, tag="ihalf")
    nc.vector.tensor_scalar(ihalf[:], ms.bitcast(dt.int32), 1, None,
                            op0=ALU.arith_shift_right)
    y0 = st.tile([128, 1], dt.float32, tag="y0")
    nc.vector.tensor_scalar(y0[:].bitcast(dt.int32), ihalf[:], -1, QUAKE,
                            op0=ALU.mult, op1=ALU.add)
    y = y0
    for it in range(3):
        t1 = st.tile([128, 1], dt.float32, tag=f"nw{it}a")
        nc.vector.tensor_tensor(t1[:], y[:], y[:], op=ALU.mult)
        t2 = st.tile([128, 1], dt.float32, tag=f"nw{it}b")
        nc.vector.tensor_tensor(t2[:], t1[:], ms, op=ALU.mult)
        t3 = st.tile([128, 1], dt.float32, tag=f"nw{it}c")
        nc.vector.tensor_scalar(t3[:], t2[:], -0.5, 1.5, op0=ALU.mult,
                                op1=ALU.add)
        y1 = st.tile([128, 1], dt.float32, tag=f"nw{it}d")
        nc.vector.tensor_tensor(y1[:], t3[:], y[:], op=ALU.mult)
        y = y1
    return y


def _quant_x_tile(nc, pools, x_t, g_bc, inv_s_dst):
    """RMSNorm + per-token int8 absmax quant of one [128, C] tile.

    Returns xq (bf16, integer-valued, [128, C]).  Writes the inverse scale
    column (= clip(amax,eps)/127) into inv_s_dst ([128,1] AP).
    sum(x^2) runs on GPSIMD, the normalize/round chain on DVE, and the final
    magic-constant subtraction on ACT — ACT itself stays exp-table-only.
    """
    sc, st = pools["scratch"], pools["stats"]
    xsq = pools.get("dump", sc).tile([128, C], dt.float32, tag="xsq")
    sumsq = st.tile([128, 1], dt.float32, tag="sumsq")
    nc.scalar.activation(xsq[:], x_t, AF.Square, accum_out=sumsq[:])
    ms = st.tile([128, 1], dt.float32, tag="ms")
    nc.vector.tensor_scalar(ms[:], sumsq[:], float(1.0 / C), EPS,
                            op0=ALU.mult, op1=ALU.add)
    rstd = _rsqrt_col(nc, st, ms[:])
    xn = sc.tile([128, C], dt.float32, tag="xn")
    nc.vector.scalar_tensor_tensor(xn[:], x_t, rstd[:], g_bc[:],
                                   op0=ALU.mult, op1=ALU.mult)
    amax = st.tile([128, 1], dt.float32, tag="amax")
    nc.vector.tensor_reduce(amax[:], xn[:], axis=AX.X, op=ALU.max,
                            apply_absolute_value=True)
    amax_c = st.tile([128, 1], dt.float32, tag="amax_c")
    nc.vector.tensor_scalar(amax_c[:], amax[:], EPS, None, op0=ALU.max)
    r_amax = st.tile([128, 1], dt.float32, tag="r_amax")
    nc.vector.reciprocal(r_amax[:], amax_c[:])
    s_col = st.tile([128, 1], dt.float32, tag="s_col")
    nc.vector.tensor_scalar(s_col[:], r_amax[:], 127.0, None, op0=ALU.mult)
    nc.vector.tensor_scalar(inv_s_dst, amax_c[:], float(1.0 / 127.0), None,
                            op0=ALU.mult)
    t_r = sc.tile([128, C], dt.float32, tag="t_r")
    nc.vector.tensor_scalar(t_r[:], xn[:], s_col[:], MAGIC,
                            op0=ALU.mult, op1=ALU.add)
    xq = pools["xq"].tile([128, C], dt.bfloat16, tag="xq")
    nc.scalar.activation(xq[:], t_r[:], AF.Copy, bias=-MAGIC, scale=1.0)
    return xq


def build_program(debug_taps=False):
    nc = bacc.Bacc("TRN2", target_bir_lowering=False, debug=False, num_devices=8)

    xb_d = nc.dram_tensor("xb", [N, C], dt.float32, kind="ExternalInput")
    xm_d = nc.dram_tensor("xm", [NQ, C], dt.float32, kind="ExternalInput")
    wq_d = nc.dram_tensor("wqkv", [3 * C, C], dt.float32, kind="ExternalInput")
    wp_d = nc.dram_tensor("wproj", [C, C], dt.float32, kind="ExternalInput")
    gq_d = nc.dram_tensor("gq", [1, C], dt.float32, kind="ExternalInput")
    gp_d = nc.dram_tensor("gp", [1, C], dt.float32, kind="ExternalInput")
    out_d = nc.dram_tensor("out", [NQ, C], dt.float32, kind="ExternalOutput")
    dbg = {}
    if debug_taps:
        for nm, shape, d in (
                ("xq0", [128, C], dt.bfloat16),
                ("inv_s_all", [128, TKV], dt.float32),
                ("inv_s_my", [128, TQ], dt.float32),
                ("wqkvT0", [128, 3 * C], dt.bfloat16),
                ("qt0", [128, NQ], dt.float32),
                ("kt0", [128, N], dt.float32),
                ("vt0", [128, H * (D + 1)], dt.float32),
                ("alpha", [128, NQ], dt.float32),
                ("avsb0", [D + 1, NQ], dt.float32),
                ("att0", [128, C], dt.float32)):
            dbg[nm] = nc.dram_tensor(f"dbg_{nm}", shape, d, kind="ExternalOutput")

    with tile.TileContext(nc) as tc, ExitStack() as ctx:
        # ---- persistent pools & constants --------------------------------
        const = ctx.enter_context(tc.tile_pool(name="const", bufs=1))
        stats = ctx.enter_context(tc.tile_pool(name="stats", bufs=6))
        wT = ctx.enter_context(tc.tile_pool(name="wT", bufs=1))
        attout_p = ctx.enter_context(tc.tile_pool(name="attout", bufs=1))

        warm = const.tile([1, 1], dt.float32)
        nc.vector.memset(warm[:], 0.0)
        warm2 = const.tile([1, 1], dt.float32)
        nc.scalar.activation(warm2[:], warm[:], AF.Square)  # act-table load @ t=0

        ones_row = const.tile([1, 128], dt.float32)
        nc.vector.memset(ones_row[:], 1.0)
        ones_col = const.tile([128, 1], dt.float32)
        nc.vector.memset(ones_col[:], 1.0)
        eps_col = const.tile([128, 1], dt.float32)
        nc.vector.memset(eps_col[:], EPS)

        iota_c = const.tile([128, 1], dt.int32)
        nc.gpsimd.iota(iota_c[:], pattern=[[0, 1]], channel_multiplier=1)
        iota_r = const.tile([128, 128], dt.int32)
        nc.gpsimd.iota(iota_r[:], pattern=[[1, 128]], channel_multiplier=0)
        iota_cf = const.tile([128, 1], dt.float32)
        nc.vector.tensor_copy(iota_cf[:], iota_c[:])
        iota_rf = const.tile([128, 128], dt.float32)
        nc.vector.tensor_copy(iota_rf[:], iota_r[:])
        ident = const.tile([128, 128], dt.float32)
        nc.vector.tensor_scalar(ident[:], iota_rf[:], iota_cf[:], None,
                                op0=ALU.is_equal)

        gq_bc = const.tile([128, C], dt.float32)
        gp_bc = const.tile([128, C], dt.float32)
        with tc.tile_pool(name="grow", bufs=1) as grow, \
             tc.tile_pool(name="bc_ps", bufs=2, space="PSUM") as bc_ps:
            gq_row = grow.tile([1, C], dt.float32)
            nc.sync.dma_start(gq_row[:], gq_d.ap())
            gp_row = grow.tile([1, C], dt.float32)
            nc.sync.dma_start(gp_row[:], gp_d.ap())
            for row, bc in ((gq_row, gq_bc), (gp_row, gp_bc)):
                for lo in (0, 512):
                    hi = min(lo + 512, C)
                    ps = bc_ps.tile([128, 512], dt.float32, tag="gbc")
                    nc.tensor.matmul(ps[:, 0:hi - lo], ones_row[:],
                                     row[:, lo:hi], start=True, stop=True)
                    nc.vector.tensor_copy(bc[:, lo:hi], ps[:, 0:hi - lo])

        inv_s_all = const.tile([128, TKV], dt.float32)
        inv_s_my = const.tile([128, TQ], dt.float32)

        # transposed quantized weights: wqkvT[p, cc, o] = wq_qkv[o, 128cc+p]
        wqkvT = wT.tile([128, NCC, 3 * C], dt.bfloat16)
        wprojT = wT.tile([128, NCC, C], dt.bfloat16)

        # ---- phase W: weight quantization --------------------------------
        # All input DMAs are emitted first (w then x) so no load ever queues
        # behind the quant-gated weight transposes on the DMA pipe.
        xstage = ctx.enter_context(tc.tile_pool(name="xstage", bufs=4))
        scales = {}  # wname -> (thr_col, nthr_col, inv_sw_col, meanc11)
        with tc.tile_pool(name="wf32", bufs=1) as wf32, \
             tc.tile_pool(name="wq_st", bufs=3) as wq_st, \
             tc.tile_pool(name="w_ps", bufs=2, space="PSUM") as w_ps:
            wtiles_all = {}
            for wname, w_d, n_big in (("q", wq_d, 3), ("p", wp_d, 1)):
                wtiles_all[wname] = []
                for g in range(n_big):
                    w_t = wf32.tile([128, NCC, C], dt.float32, name=f"w_{wname}{g}")
                    src = w_d.ap()[g * 768:(g + 1) * 768, :].rearrange(
                        "(s p) c -> p s c", p=128)
                    nc.sync.dma_start(w_t[:], src)
                    wtiles_all[wname].append(w_t)

            x_tiles = []
            for i in range(10):
                src_d, off = (xm_d, i * 256) if i < 2 else (xb_d, (i - 2) * 256)
                xt = xstage.tile([128, 2, C], dt.float32, tag="xbig",
                                 name=f"xbig{i}")
                nc.sync.dma_start(
                    xt[:], src_d.ap()[off:off + 256, :].rearrange(
                        "(s p) c -> p s c", p=128))
                x_tiles.append(xt)

            for wname, n_big, dstT in (("q", 3, wqkvT), ("p", 1, wprojT)):
                wtiles = wtiles_all[wname]
                wsums = const.tile([128, n_big], dt.float32, name=f"wsums_{wname}")
                for g in range(n_big):
                    nc.vector.tensor_reduce(wsums[:, g:g + 1], wtiles[g][:],
                                            axis=AX.XY, op=ALU.add,
                                            apply_absolute_value=True)
                colsum = const.tile([128, 1], dt.float32, name=f"colsum_{wname}")
                nc.vector.tensor_reduce(colsum[:], wsums[:], axis=AX.X, op=ALU.add)
                tot_ps = w_ps.tile([1, 1], dt.float32, tag="tot")
                nc.tensor.matmul(tot_ps[:], colsum[:], ones_col[:],
                                 start=True, stop=True)
                meanc = const.tile([1, 1], dt.float32, name=f"meanc_{wname}")
                nc.vector.tensor_scalar(meanc[:], tot_ps[:],
                                        float(1.0 / (n_big * 768 * C)), EPS,
                                        op0=ALU.mult, op1=ALU.max)
                thr11 = const.tile([1, 1], dt.float32, name=f"thr11_{wname}")
                nc.vector.tensor_scalar(thr11[:], meanc[:], 0.5, None, op0=ALU.mult)
                thr_col = const.tile([128, 1], dt.float32, name=f"thrc_{wname}")
                nthr_col = const.tile([128, 1], dt.float32, name=f"nthrc_{wname}")
                inv_sw_col = const.tile([128, 1], dt.float32, name=f"iswc_{wname}")
                for src11, dst in ((thr11, thr_col), (meanc, inv_sw_col)):
                    ps = w_ps.tile([128, 1], dt.float32, tag="bc1")
                    nc.tensor.matmul(ps[:], ones_row[:], src11[:],
                                     start=True, stop=True)
                    nc.vector.tensor_copy(dst[:], ps[:])
                nc.vector.tensor_scalar(nthr_col[:], thr_col[:], -1.0, None,
                                        op0=ALU.mult)
                scales[wname] = (thr_col, nthr_col, inv_sw_col, meanc)

                for g in range(n_big):
                    w_t = wtiles[g]
                    for sch in range(NCC):
                        sub = w_t[:, sch, :]
                        bneg = wq_st.tile([128, C], dt.bfloat16, tag="bneg")
                        nc.gpsimd.tensor_scalar(bneg[:], sub, nthr_col[:], None,
                                                op0=ALU.is_le)
                        wq_t = wq_st.tile([128, C], dt.bfloat16, tag="wq")
                        nc.vector.scalar_tensor_tensor(wq_t[:], sub, thr_col[:],
                                                       bneg[:], op0=ALU.is_ge,
                                                       op1=ALU.subtract)
                        off = g * 768 + sch * 128
                        nc.sync.dma_start(dstT[:, :, off:off + 128], wq_t[:],
                                          transpose=True)

        swsq8 = const.tile([1, 1], dt.float32)
        nc.vector.tensor_scalar(swsq8[:], scales["q"][3][:], scales["q"][3][:],
                                0.125, op0=ALU.mult, op1=ALU.mult)

        # ---- phase X + M1 ------------------------------------------------
        xqT_pool = ctx.enter_context(tc.tile_pool(name="xqTall", bufs=1))
        xqT = xqT_pool.tile([128, NCC, N], dt.bfloat16)       # [p, cc, tok]
        xqTm = xqT_pool.tile([128, NCC, NQ], dt.bfloat16)
        v_p = ctx.enter_context(tc.tile_pool(name="v", bufs=1))
        qt_p = ctx.enter_context(tc.tile_pool(name="qt", bufs=1))
        vt = [v_p.tile([128, H * (D + 1)], dt.float32r, name=f"vt{t}")
              for t in range(TKV)]
        qt = [qt_p.tile([128, NQ], dt.float32r, name=f"qt{f}")
              for f in range(NCC)]
        alpha_bc = const.tile([128, NQ], dt.float32)

        with tc.tile_pool(name="xscratch", bufs=2) as xscratch, \
             tc.tile_pool(name="xdump", bufs=1) as xdump, \
             tc.tile_pool(name="xqst", bufs=3) as xqst, \
             tc.tile_pool(name="m1_ps", bufs=3, space="PSUM") as m1_ps, \
             tc.tile_pool(name="al_ps", bufs=1, space="PSUM") as al_ps:
            pools = {"scratch": xscratch, "dump": xdump, "stats": stats,
                     "xq": xqst, "eps_col": eps_col}

            # -- my-token chunks first (enables Q path early) --
            for tg in range(2):
                xm_big = x_tiles[tg]
                for tt in range(2):
                    t = tg * 2 + tt
                    xq = _quant_x_tile(nc, pools, xm_big[:, tt, :], gq_bc,
                                       inv_s_my[:, t:t + 1])
                    nc.sync.dma_start(xqTm[:, :, t * 128:(t + 1) * 128], xq[:],
                                        transpose=True)

            # alpha = inv_s_my * inv_sw^2/8, as a [128, NQ] broadcast tile
            tp = al_ps.tile([TQ, 128], dt.float32, tag="alT")
            nc.tensor.transpose(tp[:], inv_s_my[:], ident[:])
            al4 = xdump.tile([TQ, 128], dt.float32)
            nc.vector.tensor_copy(al4[:], tp[:])
            alrow = xdump.tile([1, NQ], dt.float32)
            for t in range(TQ):
                nc.sync.dma_start(alrow[:, t * 128:(t + 1) * 128], al4[t:t + 1, :])
            alrow2 = xdump.tile([1, NQ], dt.float32)
            nc.vector.tensor_scalar(alrow2[:], alrow[:], swsq8[:], None,
                                    op0=ALU.mult)
            ps = al_ps.tile([128, NQ], dt.float32, tag="alT")
            nc.tensor.matmul(ps[:], ones_row[:], alrow2[:], start=True, stop=True)
            nc.vector.tensor_copy(alpha_bc[:], ps[:])

            # Q^T [feat, tok] with fused alpha scale
            for f in range(NCC):
                ps = m1_ps.tile([128, NQ], dt.float32, tag="m1")
                for c in range(NCC):
                    nc.tensor.matmul(ps[:], wqkvT[:, c, f * 128:(f + 1) * 128],
                                     xqTm[:, c, :],
                                     start=(c == 0), stop=(c == NCC - 1))
                nc.vector.tensor_tensor(qt[f][:], ps[:], alpha_bc[:], op=ALU.mult)
                if debug_taps and f == 0:
                    nc.sync.dma_start(dbg["qt0"].ap(), qt[f][:].bitcast(dt.float32))

            # -- kv chunks, V matmuls interleaved --
            for tg in range(8):
                xb_big = x_tiles[2 + tg]
                for tt in range(2):
                    t = tg * 2 + tt
                    xq = _quant_x_tile(nc, pools, xb_big[:, tt, :], gq_bc,
                                       inv_s_all[:, t:t + 1])
                    if debug_taps and t == 0:
                        nc.sync.dma_start(dbg["xq0"].ap(), xq[:])
                    nc.sync.dma_start(xqT[:, :, t * 128:(t + 1) * 128], xq[:],
                                        transpose=True)
                    # per-chunk V scale column
                    vcol = stats.tile([128, 1], dt.float32, tag="vcol")
                    nc.vector.tensor_scalar(vcol[:], inv_s_all[:, t:t + 1],
                                            scales["q"][2][:], None, op0=ALU.mult)
                    v_re = vt[t][:].rearrange("p (h x) -> p h x", x=D + 1)
                    for half in range(2):
                        ps_full = m1_ps.tile([128, 512], dt.float32, tag="m1", name="psv")
                        ps = ps_full[:, 0:384]
                        for c in range(NCC):
                            nc.tensor.matmul(
                                ps[:], xqT[:, c, t * 128:(t + 1) * 128],
                                wqkvT[:, c, 2 * C + half * 384:2 * C + (half + 1) * 384],
                                start=(c == 0), stop=(c == NCC - 1))
                        nc.vector.tensor_scalar(
                            v_re[:, 6 * half:6 * half + 6, 0:D],
                            ps[:].rearrange("p (h x) -> p h x", x=D),
                            vcol[:], None, op0=ALU.mult)
                    nc.vector.memset(v_re[:, :, D:D + 1].bitcast(dt.float32), 1.0)
            if debug_taps:
                nc.sync.dma_start(dbg["vt0"].ap(), vt[0][:].bitcast(dt.float32))
                nc.sync.dma_start(dbg["inv_s_all"].ap(), inv_s_all[:])
                nc.sync.dma_start(dbg["inv_s_my"].ap(), inv_s_my[:])
                nc.sync.dma_start(dbg["alpha"].ap(), alpha_bc[:])
                nc.sync.dma_start(dbg["wqkvT0"].ap(), wqkvT[:, 0, :])

        # ---- phase A: attention (lazy K^T per feature chunk) -------------
        att_out = [attout_p.tile([128, C], dt.float32, name=f"ao{t}")
                   for t in range(TQ)]
        with tc.tile_pool(name="k_ps", bufs=1, space="PSUM") as k_ps, \
             tc.tile_pool(name="s_ps", bufs=2, space="PSUM") as s_ps, \
             tc.tile_pool(name="av_ps", bufs=1, space="PSUM") as av_ps, \
             tc.tile_pool(name="tp_ps", bufs=1, space="PSUM") as tp_ps, \
             tc.tile_pool(name="ktroll", bufs=2) as ktroll, \
             tc.tile_pool(name="aexp", bufs=2) as aexp, \
             tc.tile_pool(name="avsb", bufs=2) as avsb:
            def build_kt_block(ktf, f, t):
                ps = k_ps.tile([128, 512], dt.float32, tag="k", name="kps")
                for c in range(NCC):
                    nc.tensor.matmul(
                        ps[:], wqkvT[:, c, C + f * 128:C + (f + 1) * 128],
                        xqT[:, c, t * 512:(t + 1) * 512],
                        start=(c == 0), stop=(c == NCC - 1))
                nc.vector.tensor_copy(ktf[:, t * 512:(t + 1) * 512], ps[:])

            kt_cur = ktroll.tile([128, N], dt.float32r, tag="kt", name="kt0t")
            for t in range(4):
                build_kt_block(kt_cur, 0, t)
            if debug_taps:
                nc.sync.dma_start(dbg["kt0"].ap(), kt_cur[:].bitcast(dt.float32))
            def emit_qk_pair(sp, ktf, f, cch):
                # both heads of pair f share the kv chunk: head 2f into
                # columns 0:512, head 2f+1 into 512:1024 (separate banks)
                for hi, po in ((0, 0), (1, 64)):
                    nc.tensor.matmul(
                        sp[:, hi * NQ:(hi + 1) * NQ],
                        ktf[po:po + 64, cch * 128:(cch + 1) * 128],
                        qt[f][po:po + 64, :], start=True, stop=True)

            for f in range(NCC):
                ktf = kt_cur
                kt_next = None
                # one exp call covers BOTH heads of the pair (same kv chunk
                # -> same per-partition scale): halves ACT call count.
                av0 = av_ps.tile([D + 1, NQ], dt.float32, tag="av0")
                av1 = av_ps.tile([D + 1, NQ], dt.float32, tag="av1")
                sps = [s_ps.tile([128, 2 * NQ], dt.float32, tag="s", name="sp0")]
                emit_qk_pair(sps[0], ktf, f, 0)
                for cch in range(TKV):
                    if cch + 1 < TKV:
                        sp1 = s_ps.tile([128, 2 * NQ], dt.float32, tag="s",
                                        name="sp1")
                        emit_qk_pair(sp1, ktf, f, cch + 1)
                        sps.append(sp1)
                    if f + 1 < NCC and cch % 4 == 1:
                        if kt_next is None:
                            kt_next = ktroll.tile([128, N], dt.float32r,
                                                  tag="kt", name="ktn")
                        build_kt_block(kt_next, f + 1, cch // 4)
                    ae = aexp.tile([128, 2 * NQ], dt.float32r, tag="ae")
                    nc.scalar.activation(ae[:], sps[cch][:], AF.Exp,
                                         scale=inv_s_all[:, cch:cch + 1])
                    for hi, av in ((0, av0), (1, av1)):
                        h = 2 * f + hi
                        nc.tensor.matmul(
                            av[:], vt[cch][:, h * (D + 1):(h + 1) * (D + 1)],
                            ae[:, hi * NQ:(hi + 1) * NQ],
                            start=(cch == 0), stop=(cch == TKV - 1))
                for hi, av in ((0, av0), (1, av1)):
                    h = 2 * f + hi
                    av_sb = avsb.tile([D + 1, NQ], dt.float32, tag="avsb")
                    nc.vector.tensor_copy(av_sb[:], av[:])
                    if debug_taps and h == 0:
                        nc.sync.dma_start(dbg["avsb0"].ap(), av_sb[:])
                    for t in range(TQ):
                        tp = tp_ps.tile([128, D + 1], dt.float32, tag="tp")
                        nc.tensor.transpose(tp[:], av_sb[:, t * 128:(t + 1) * 128],
                                            ident[0:D + 1, 0:D + 1])
                        dcol = stats.tile([128, 1], dt.float32, tag="dcol")
                        nc.vector.reciprocal(dcol[:], tp[:, D:D + 1])
                        nc.vector.tensor_scalar(att_out[t][:, h * D:(h + 1) * D],
                                                tp[:, 0:D], dcol[:], None,
                                                op0=ALU.mult)
                kt_cur = kt_next

        # ---- phase P: proj bitlinear -------------------------------------
        with tc.tile_pool(name="p_scr", bufs=1) as p_scr, \
             tc.tile_pool(name="p_dump", bufs=1) as p_dump, \
             tc.tile_pool(name="p_xq", bufs=2) as p_xq, \
             tc.tile_pool(name="xq2T", bufs=1) as xq2T_p, \
             tc.tile_pool(name="m2_ps", bufs=3, space="PSUM") as m2_ps, \
             tc.tile_pool(name="outsb", bufs=2) as outsb:
            xq2T = xq2T_p.tile([128, NCC, NQ], dt.bfloat16)
            pools2 = {"scratch": p_scr, "dump": p_dump, "stats": stats,
                      "xq": p_xq, "eps_col": eps_col}
            inv_s2 = const.tile([128, TQ], dt.float32)
            if debug_taps:
                nc.sync.dma_start(dbg["att0"].ap(), att_out[0][:])
            for t in range(TQ):
                xq2 = _quant_x_tile(nc, pools2, att_out[t][:], gp_bc,
                                    inv_s2[:, t:t + 1])
                nc.sync.dma_start(xq2T[:, :, t * 128:(t + 1) * 128], xq2[:],
                                      transpose=True)
            pcol = const.tile([128, TQ], dt.float32)
            nc.vector.tensor_scalar(pcol[:], inv_s2[:], scales["p"][2][:],
                                    None, op0=ALU.mult)
            for t in range(TQ):
                o_sb = outsb.tile([128, C], dt.float32, tag="osb")
                for half in range(2):
                    ps = m2_ps.tile([128, 384], dt.float32, tag="m2")
                    for c in range(NCC):
                        nc.tensor.matmul(
                            ps[:], xq2T[:, c, t * 128:(t + 1) * 128],
                            wprojT[:, c, half * 384:(half + 1) * 384],
                            start=(c == 0), stop=(c == NCC - 1))
                    nc.scalar.mul(o_sb[:, half * 384:(half + 1) * 384],
                                  ps[:], pcol[:, t:t + 1])
                nc.sync.dma_start(out_d.ap()[t * 128:(t + 1) * 128, :], o_sb[:])

    nc.compile()
    return nc


def _get_program(debug_taps=False):
    key = ("nc", debug_taps)
    if key not in _CACHE:
        _CACHE[key] = build_program(debug_taps)
    return _CACHE[key]


def kernel(x, w_qkv, g_qkv, w_proj, g_proj, _trace=False, _debug_taps=False,
           **trace_kwargs):
    x = np.ascontiguousarray(np.asarray(x, dtype=np.float32))
    w_qkv = np.ascontiguousarray(np.asarray(w_qkv, dtype=np.float32))
    w_proj = np.ascontiguousarray(np.asarray(w_proj, dtype=np.float32))
    gq = np.ascontiguousarray(np.asarray(g_qkv, dtype=np.float32).reshape(1, C))
    gp = np.ascontiguousarray(np.asarray(g_proj, dtype=np.float32).reshape(1, C))

    nc = _get_program(_debug_taps)
    in_maps = []
    for core in range(8):
        b, j = core // 4, core % 4
        in_maps.append({
            "xb": x[b],
            "xm": x[b, j * NQ:(j + 1) * NQ],
            "wqkv": w_qkv,
            "wproj": w_proj,
            "gq": gq,
            "gp": gp,
        })
    res = run_bass_kernel_spmd(nc, in_maps, list(range(8)), trace=_trace,
                               **trace_kwargs)
    out = np.empty((B, N, C), dtype=np.float32)
    for core in range(8):
        b, j = core // 4, core % 4
        out[b, j * NQ:(j + 1) * NQ] = res.results[core]["out"]
    if _trace or _debug_taps:
        return out, res
    return out



# revision 87
# speedup vs baseline: 1.0802x; 1.0802x over previous
"""BitNet attention SPMD kernel for 8 Trainium2 NeuronCores — v2 (tensor parallel).

Problem: nn_BitNetAttention (B=2, N=2048, C=768, H=12, D=64).

Sharding (per the hint): data-parallel over batch (2 groups of 4 cores) x
tensor-parallel over heads within a group (3 heads per core, column-parallel
qkv).  Each core quantizes its batch's 2048 tokens ONCE (the v1 kernel
re-derived K/V 4x per group), computes Q/K/V only for its 3 heads, runs the
full N x N attention slab for those heads, and the group exchanges attention
outputs with a masked ReduceScatter so every core projects its own 512-token
slice with an exact full-C RMSNorm.  Two collectives total:

- AllReduce [1,2]: the BitNet per-tensor weight-quant scale needs mean|w| over
  the FULL w_qkv / w_proj; each core reduces only its head-slice (w_qkv) and
  the whole w_proj, and the group sums.  Overlaps the x-quant phase.
- ReduceScatter [2048,768]->[512,768]: core g's attention slab occupies
  columns 192g:192g+192 of the gathered [N, C]; a one-hot mask input (host
  supplies onehot(g), keeping the SPMD program core-id-free) places each
  core's slab before the sum.

Numerics: quantized matmuls run as exact integer arithmetic in bf16 (int8
activations x ternary weights, fp32 PSUM).  Attention matmuls run float32r.
Per-token dequant scales fold into the Q^T/K^T PSUM->SBUF copies (free-dim
broadcast rows), so softmax exp needs no per-partition scale and batches two
KV tiles per ACT call.  Softmax denominators ride along as a ones-column in V.
"""
import sys
sys.path.insert(0, "/opt/trn_rl_repo")

import numpy as np
from contextlib import ExitStack

import concourse.bass as bass
import concourse.mybir as mybir
import concourse.tile as tile
import concourse.bacc as bacc
from concourse.bass_utils import run_bass_kernel_spmd

dt = mybir.dt
AF = mybir.ActivationFunctionType
ALU = mybir.AluOpType
AX = mybir.AxisListType

B, N, C = 2, 2048, 768
H, D = 12, 64
HG = 3                   # heads per core
CQ = HG * D              # 192 attention-output cols per core
NT = N // 128            # 16 token tiles
NCC = C // 128           # 6 contraction chunks
QB = 4                   # q blocks of 512
NQ = N // 4              # 512 output tokens per core
WQR = 3 * CQ             # 576 rows of w_qkv owned per core
EPS = 1e-5
MAGIC = 12582912.0       # 1.5*2^23: x+MAGIC lands where ulp=1 (rounds x)
QUAKE = 0x5F3759DF
# bf16 Schraudolph: bitcast_bf16(i16(A*x + B)) ~ exp(x); measured on-device
# rel err is mean-zero, range [-4.2%, +2.2%].
EXPA16 = float(2 ** 7 / np.log(2))
EXPB16 = float(127 * 2 ** 7 - 486411.0 / 65536)

_CACHE = {}


def _rsqrt_tile(nc, pool, ms, w):
    """1/sqrt(ms) for a [128, w] tile on DVE (bit-trick seed + 3 Newton)."""
    ihalf = pool.tile([128, w], dt.int32, tag="ihalf")
    nc.vector.tensor_scalar(ihalf[:], ms.bitcast(dt.int32), 1, None,
                            op0=ALU.arith_shift_right)
    y0 = pool.tile([128, w], dt.float32, tag="y0")
    nc.vector.tensor_scalar(y0[:].bitcast(dt.int32), ihalf[:], -1, QUAKE,
                            op0=ALU.mult, op1=ALU.add)
    y = y0
    for it in range(3):
        t1 = pool.tile([128, w], dt.float32, tag=f"nw{it}a")
        nc.vector.tensor_tensor(t1[:], y[:], y[:], op=ALU.mult)
        t2 = pool.tile([128, w], dt.float32, tag=f"nw{it}b")
        nc.vector.tensor_tensor(t2[:], t1[:], ms, op=ALU.mult)
        t3 = pool.tile([128, w], dt.float32, tag=f"nw{it}c")
        nc.vector.tensor_scalar(t3[:], t2[:], -0.5, 1.5, op0=ALU.mult,
                                op1=ALU.add)
        y1 = pool.tile([128, w], dt.float32, tag=f"nw{it}d")
        nc.vector.tensor_tensor(y1[:], t3[:], y[:], op=ALU.mult)
        y = y1
    return y


def build_program(g_is_one=True, debug_taps=False):
    nc = bacc.Bacc("TRN2", target_bir_lowering=False, debug=False,
                   num_devices=8)

    xb_d = nc.dram_tensor("xb", [N, C], dt.float32, kind="ExternalInput")
    wqs_d = nc.dram_tensor("wqs", [WQR, C], dt.float32, kind="ExternalInput")
    wp_d = nc.dram_tensor("wp", [C, C], dt.float32, kind="ExternalInput")
    gq_d = nc.dram_tensor("gq", [1, C], dt.float32, kind="ExternalInput")
    gp_d = nc.dram_tensor("gp", [1, C], dt.float32, kind="ExternalInput")
    mask_d = nc.dram_tensor("mask", [1, 4], dt.float32, kind="ExternalInput")
    out_d = nc.dram_tensor("out", [NQ, C], dt.float32, kind="ExternalOutput")
    dbg = {}
    if debug_taps:
        for nm, shape, d in (
                ("xq0", [128, C], dt.bfloat16),
                ("inv_s", [128, NT], dt.float32),
                ("qt01", [128, N], dt.float32),
                ("kt01", [128, N], dt.float32),
                ("vt0", [128, HG * (D + 1)], dt.float32),
                ("ae0", [128, 1024], dt.float32),
                ("att0", [128, 4 * CQ], dt.float32),
                ("recv0", [128, C], dt.bfloat16)):
            dbg[nm] = nc.dram_tensor(f"dbg_{nm}", shape, d,
                                     kind="ExternalOutput")

    with tile.TileContext(nc) as tc, ExitStack() as ctx:
        const = ctx.enter_context(tc.tile_pool(name="const", bufs=1))
        stats = ctx.enter_context(tc.tile_pool(name="stats", bufs=6))
        wT = ctx.enter_context(tc.tile_pool(name="wT", bufs=1))
        dram = ctx.enter_context(tc.tile_pool(name="dram", bufs=1,
                                              space="DRAM"))

        warm = const.tile([1, 1], dt.float32)
        nc.vector.memset(warm[:], 0.0)
        warm2 = const.tile([1, 1], dt.float32)
        nc.scalar.activation(warm2[:], warm[:], AF.Square)  # act table @ t=0

        ones_row = const.tile([1, 128], dt.float32)
        nc.vector.memset(ones_row[:], 1.0)
        ones_col = const.tile([128, 1], dt.float32)
        nc.vector.memset(ones_col[:], 1.0)

        iota_c = const.tile([128, 1], dt.int32)
        nc.gpsimd.iota(iota_c[:], pattern=[[0, 1]], channel_multiplier=1)
        iota_r = const.tile([128, 128], dt.int32)
        nc.gpsimd.iota(iota_r[:], pattern=[[1, 128]], channel_multiplier=0)
        iota_cf = const.tile([128, 1], dt.float32)
        nc.vector.tensor_copy(iota_cf[:], iota_c[:])
        iota_rf = const.tile([128, 128], dt.float32)
        nc.vector.tensor_copy(iota_rf[:], iota_r[:])
        ident = const.tile([128, 128], dt.float32)
        nc.vector.tensor_scalar(ident[:], iota_rf[:], iota_cf[:], None,
                                op0=ALU.is_equal)

        mask_sb = const.tile([1, 4], dt.float32)
        nc.sync.dma_start(mask_sb[:], mask_d.ap())
        mask_bc = const.tile([128, 4], dt.float32)
        with tc.tile_pool(name="mk_ps", bufs=1, space="PSUM") as mk_ps:
            ps = mk_ps.tile([128, 4], dt.float32, tag="mk")
            nc.tensor.matmul(ps[:], ones_row[:], mask_sb[:], start=True,
                             stop=True)
            nc.vector.tensor_copy(mask_bc[:], ps[:])

        # broadcast gains (general-g path only)
        if not g_is_one:
            gq_bc = const.tile([128, C], dt.float32)
            gp_bc = const.tile([128, C], dt.float32)
            with tc.tile_pool(name="grow", bufs=1) as grow, \
                 tc.tile_pool(name="bc_ps", bufs=2, space="PSUM") as bc_ps:
                gq_row = grow.tile([1, C], dt.float32)
                nc.sync.dma_start(gq_row[:], gq_d.ap())
                gp_row = grow.tile([1, C], dt.float32)
                nc.sync.dma_start(gp_row[:], gp_d.ap())
                for row, bc in ((gq_row, gq_bc), (gp_row, gp_bc)):
                    for lo in (0, 512):
                        hi = min(lo + 512, C)
                        ps = bc_ps.tile([128, 512], dt.float32, tag="gbc")
                        nc.tensor.matmul(ps[:, 0:hi - lo], ones_row[:],
                                         row[:, lo:hi], start=True, stop=True)
                        nc.vector.tensor_copy(bc[:, lo:hi], ps[:, 0:hi - lo])

        # ---- long-lived pools/tiles (opened before staging: LIFO close) ---
        wfp = ctx.enter_context(tc.tile_pool(name="wf32", bufs=1))
        wpf = wfp.tile([128, NCC, C], dt.float32)     # wproj
        xqT_pool = ctx.enter_context(tc.tile_pool(name="xqT", bufs=1))
        xqT = xqT_pool.tile([128, NCC, N], dt.float16)
        wqT = wT.tile([128, NCC, WQR], dt.float16)
        wpT = wT.tile([128, NCC, C], dt.bfloat16)
        tern = ctx.enter_context(tc.tile_pool(name="tern", bufs=4))
        NTF = 16                 # kv tiles 0:NTF -> ACT exp (f32r ae/V);
        #                          rest -> DVE bf16-Schraudolph
        qkt_p = ctx.enter_context(tc.tile_pool(name="qkt", bufs=1))
        tq01 = qkt_p.tile([128, N], dt.float32r)
        tq2 = qkt_p.tile([64, N], dt.float32r)
        tk01 = qkt_p.tile([128, N], dt.float32r)
        tk2 = qkt_p.tile([64, N], dt.float32r)
        v_p = ctx.enter_context(tc.tile_pool(name="v", bufs=1))
        vtf = v_p.tile([128, NTF, HG * (D + 1)], dt.float32r)
        nc.vector.memset(vtf[:].bitcast(dt.float32), 1.0)
        vtb = None
        if NT > NTF:
            vtb = v_p.tile([128, NT - NTF, HG * (D + 1)], dt.bfloat16)
            nc.vector.memset(vtb[:], 1.0)
        wsc = ctx.enter_context(tc.tile_pool(name="wsc", bufs=2))

        # ---------------- DMA-in: weights then x ---------------------------
        # staging pools released before attention (stage_es.close())
        stage_es = ExitStack()
        xstage = stage_es.enter_context(tc.tile_pool(name="xstage", bufs=4))
        wqfp = stage_es.enter_context(tc.tile_pool(name="wqf32", bufs=1))
        wqf = wqfp.tile([128, 4, C], dt.float32)      # wqs rows 0:512
        wqf_t = wqfp.tile([64, C], dt.float32)        # wqs rows 512:576
        nc.scalar.dma_start(wqf[:], wqs_d.ap()[0:512, :].rearrange(
            "(s p) c -> p s c", p=128))
        nc.scalar.dma_start(wqf_t[:], wqs_d.ap()[512:576, :])
        # only the first two x chunks are queued ahead of the weight-scale
        # path; the rest (and wproj) are emitted after it so the tiny
        # AllGather bounce DMAs don't queue behind ~25us of bulk transfers
        # (the cost model serializes all DMA through one engine pool).
        x_tiles = []
        for i in range(8):
            xt = xstage.tile([128, 2, C], dt.float32, tag="xbig",
                             name=f"xbig{i}")
            if i < 2:
                nc.sync.dma_start(xt[:], xb_d.ap()[i * 256:(i + 1) * 256, :]
                                  .rearrange("(s p) c -> p s c", p=128))
            x_tiles.append(xt)

        # ---------------- weight scale partials + AllReduce ----------------
        thr_q = const.tile([128, 1], dt.float32)
        nthr_q = const.tile([128, 1], dt.float32)
        thr_p = const.tile([128, 1], dt.float32)
        nthr_p = const.tile([128, 1], dt.float32)
        meanc_q = const.tile([1, 1], dt.float32)
        meanc_p = const.tile([1, 1], dt.float32)
        meanc_q_col = const.tile([128, 1], dt.float32)
        meanc_p_col = const.tile([128, 1], dt.float32)
        swsq8 = const.tile([1, 1], dt.float32)
        # Two small AllGathers (cheaper than AllReduce in the cost model):
        # #1 covers the w_qkv head-slice partials (gates the q ternarize,
        # issued as soon as the slice lands); #2 covers w_proj (only needed
        # by the deferred wproj ternarize during attention).
        def scale_issue(cols, name, q_in):
            """Partial-sum + AllGather issue; returns the gathered dram
            tile.  Collection (scale_collect) can be emitted much later so
            the read-back's SEQ-blocking wait lands on a queue with nothing
            urgent behind it."""
            ps_pool = tc.tile_pool(name=f"wsps_{name}", bufs=1, space="PSUM")
            with ps_pool as ws_ps:
                ps = ws_ps.tile([1, 1], dt.float32, tag="w")
                for i, col in enumerate(cols):
                    npart = col.shape[0]
                    nc.tensor.matmul(ps[:], col[:], ones_col[0:npart, :],
                                     start=(i == 0),
                                     stop=(i == len(cols) - 1))
                red_sb = wsc.tile([1, 1], dt.float32, name=f"red_{name}")
                nc.vector.tensor_copy(red_sb[:], ps[:])
            red_in = dram.tile([1, 1], dt.float32, name=f"ri_{name}")
            red_out = dram.tile([4, 1], dt.float32, name=f"ro_{name}")
            q_in.dma_start(red_in[:], red_sb[:])
            nc.gpsimd.collective_compute(
                "AllGather", ALU.bypass,
                replica_groups=[[0, 1, 2, 3], [4, 5, 6, 7]],
                ins=[red_in.opt()], outs=[red_out.opt()])
            return red_out, red_sb

        def scale_collect(red_out, name, q_out, denom, m11, mcol, thr,
                          nthr):
            red4_sb = wsc.tile([4, 1], dt.float32, name=f"r4_{name}")
            q_out.dma_start(red4_sb[:], red_out[:])
            with tc.tile_pool(name=f"wsps2_{name}", bufs=1,
                              space="PSUM") as ws_ps:
                rsum = ws_ps.tile([1, 1], dt.float32, tag="w")
                nc.tensor.matmul(rsum[:], ones_col[0:4, :], red4_sb[:],
                                 start=True, stop=True)
                nc.vector.tensor_scalar(m11[:], rsum[:],
                                        float(1.0 / denom), EPS,
                                        op0=ALU.mult, op1=ALU.max)
                ps = ws_ps.tile([128, 1], dt.float32, tag="w")
                nc.tensor.matmul(ps[:], ones_row[:], m11[:], start=True,
                                 stop=True)
                nc.vector.tensor_copy(mcol[:], ps[:])
                nc.vector.tensor_scalar(thr[:], ps[:], 0.5, None,
                                        op0=ALU.mult)
                nc.vector.tensor_scalar(nthr[:], ps[:], -0.5, None,
                                        op0=ALU.mult)

        with tc.tile_pool(name="wdump", bufs=1) as wdump:
            dump_q = wdump.tile([128, 4, C], dt.float32)
            col_q = wsc.tile([128, 1], dt.float32)
            nc.scalar.activation(dump_q[:], wqf[:], AF.Abs,
                                 accum_out=col_q[:])
            dump_t = wdump.tile([64, C], dt.float32, name="dump_t")
            col_t = wsc.tile([64, 1], dt.float32, name="col_t")
            nc.scalar.activation(dump_t[:], wqf_t[:], AF.Abs,
                                 accum_out=col_t[:])
            # bounce DMA on the gpsimd queue (nothing urgent behind it);
            # bulk x/wproj loads are emitted after the collective so the
            # shared DMA-engine FIFO serves the tiny transfer promptly
            ro_q, red_q_sb = scale_issue([col_q, col_t], "q", nc.gpsimd)
            # DMA FIFO orders by *issue* time, and dep-free dma_starts all
            # issue at t~0.  Touch each staging tile with a copy that reads
            # the scale bounce, so the bulk loads issue (and enter the FIFO)
            # only after the tiny collective DMA is in flight.
            for i in range(2, 8):
                nc.vector.tensor_copy(x_tiles[i][0:1, 0, 0:1],
                                      red_q_sb[:])
                nc.sync.dma_start(x_tiles[i][:],
                                  xb_d.ap()[i * 256:(i + 1) * 256, :]
                                  .rearrange("(s p) c -> p s c", p=128))
            nc.sync.dma_start(wpf[:], wp_d.ap().rearrange(
                "(s p) c -> p s c", p=128))

        # ---------------- x quantization (16 tiles) ------------------------
        sumsq_all = const.tile([128, NT], dt.float32)
        amax_c_all = const.tile([128, NT], dt.float32)
        inv_s_all = const.tile([128, NT], dt.float32)
        swsq8_col = const.tile([128, 1], dt.float32)
        xq_int = {}

        def ternarize(src, shape, thr, nthr, wdt=dt.bfloat16):
            bneg = tern.tile(shape, wdt, tag="bneg")
            nc.gpsimd.tensor_scalar(bneg[:], src, nthr[0:shape[0], :],
                                    None, op0=ALU.is_le)
            wq_t = tern.tile(shape, wdt, tag="wqt")
            nc.vector.scalar_tensor_tensor(wq_t[:], src,
                                           thr[0:shape[0], :], bneg[:],
                                           op0=ALU.is_ge,
                                           op1=ALU.subtract)
            return wq_t

        def tern_wq():
            # wq ternarize; emitted after quant tile 13 so the DVE stream
            # reaches it right as the scale AllGather result lands.
            for s in range(4):
                wq_t = ternarize(wqf[:, s, :], [128, C], thr_q, nthr_q,
                                 wdt=dt.float16)
                nc.scalar.dma_start(wqT[:, :, s * 128:(s + 1) * 128],
                                    wq_t[:], transpose=True)
            wq_tt = ternarize(wqf_t[:], [64, C], thr_q, nthr_q,
                              wdt=dt.float16)
            nc.scalar.dma_start(wqT[:, :, 512:576], wq_tt[:], transpose=True)

        with tc.tile_pool(name="qscr", bufs=4) as qscr, \
             tc.tile_pool(name="qdump", bufs=2) as qdump:
            for t in range(NT):
                x_t = x_tiles[t // 2][:, t % 2, :]
                if not g_is_one:
                    xg = qscr.tile([128, C], dt.float32, tag="xg")
                    nc.vector.tensor_tensor(xg[:], x_t, gq_bc[:],
                                            op=ALU.mult)
                    x_in = xg[:]
                else:
                    x_in = x_t
                xsq = qdump.tile([128, C], dt.float32, tag="xsq")
                nc.scalar.activation(xsq[:], x_t, AF.Square,
                                     accum_out=sumsq_all[:, t:t + 1])
                amax = stats.tile([128, 1], dt.float32, tag="amax")
                nc.vector.tensor_reduce(amax[:], x_in, axis=AX.X,
                                        op=ALU.max,
                                        apply_absolute_value=True)
                nc.vector.tensor_scalar(amax_c_all[:, t:t + 1], amax[:],
                                        EPS, None, op0=ALU.max)
                r_amax = stats.tile([128, 1], dt.float32, tag="r_amax")
                nc.vector.reciprocal(r_amax[:], amax_c_all[:, t:t + 1])
                s_col = stats.tile([128, 1], dt.float32, tag="s_col")
                nc.vector.tensor_scalar(s_col[:], r_amax[:], 127.0, None,
                                        op0=ALU.mult)
                t_r = qscr.tile([128, C], dt.float32, tag="t_r")
                nc.vector.tensor_scalar(t_r[:], x_in, s_col[:], MAGIC,
                                        op0=ALU.mult, op1=ALU.add)
                xq = qscr.tile([128, C], dt.bfloat16, tag="xq")
                nc.vector.tensor_scalar(xq[:], t_r[:], -MAGIC, None,
                                        op0=ALU.add)
                xq_int[t] = xq
                if debug_taps and t == 0:
                    nc.sync.dma_start(dbg["xq0"].ap(), xq[:])
                if t % 4 == 3:
                    # per-4-tile dequant scales, then pre-scale the int8
                    # values by inv_s into fp16 (exact to ~5e-4): the scale
                    # factors out of every integer contraction, so Q/K/V
                    # copies need no per-token broadcast tiles at all.
                    t0 = t - 3
                    sl = slice(t0, t + 1)
                    ms = stats.tile([128, 4], dt.float32, tag="ms")
                    nc.vector.tensor_scalar(ms[:], sumsq_all[:, sl],
                                            float(1.0 / C), EPS,
                                            op0=ALU.mult, op1=ALU.add)
                    rstd = _rsqrt_tile(nc, qdump, ms[:], 4)
                    pre = stats.tile([128, 4], dt.float32, tag="pre")
                    nc.vector.tensor_tensor(pre[:], amax_c_all[:, sl],
                                            rstd[:], op=ALU.mult)
                    nc.vector.tensor_scalar(inv_s_all[:, sl], pre[:],
                                            float(1.0 / 127.0), None,
                                            op0=ALU.mult)
                    for tt in range(t0, t + 1):
                        xqp = qscr.tile([128, C], dt.float16, tag="xqp")
                        nc.vector.tensor_scalar(
                            xqp[:], xq_int[tt][:],
                            inv_s_all[:, tt:tt + 1], None, op0=ALU.mult)
                        nc.scalar.dma_start(
                            xqT[:, :, tt * 128:(tt + 1) * 128], xqp[:],
                            transpose=True)
            # collect the q-scale AllGather (done well before quant ends,
            # so the ternarize below starts stall-free) and ternarize wq
            scale_collect(ro_q, "q", nc.sync, 3 * C * C, meanc_q,
                          meanc_q_col, thr_q, nthr_q)
            nc.vector.tensor_scalar(swsq8[:], meanc_q[:],
                                    meanc_q[:], 0.125,
                                    op0=ALU.mult, op1=ALU.mult)
            # [128,1] broadcast of meanc^2/8 for the exp scale
            nc.vector.tensor_scalar(swsq8_col[:], meanc_q_col[:],
                                    meanc_q_col[:], 0.125,
                                    op0=ALU.mult, op1=ALU.mult)
            tern_wq()
            if debug_taps:
                nc.sync.dma_start(dbg["inv_s"].ap(), inv_s_all[:])

        # wproj scale path (emitted post-quant so the ACT stream is not
        # blocked waiting for the late wproj load; result needed only by
        # the drip-fed wproj ternarize during attention)
        with tc.tile_pool(name="wdump2", bufs=1) as wdump2:
            dump_p = wdump2.tile([128, NCC, C], dt.float32, name="dump_p")
            col_p = wsc.tile([128, 1], dt.float32, name="col_p")
            nc.scalar.activation(dump_p[:], wpf[:], AF.Abs,
                                 accum_out=col_p[:])
            ro_p, _ = scale_issue([col_p], "p", nc.gpsimd)
            scale_collect(ro_p, "p", nc.gpsimd, 4 * C * C, meanc_p,
                          meanc_p_col, thr_p, nthr_p)

        # ---------------- QKV matmuls --------------------------------------
        # Q^T/K^T tiles [feat-part, tok-free]; rows of wqT: Q 0:192, K
        # 192:384, V 384:576.  Head packing: T01 = heads {0@p0, 1@p64},
        # T2 = head 2 @ p0 (64 partitions).  K and V first (attention for
        # q-block qb needs all K/V but only Q(qb)); Q(qb) is emitted just
        # before its attention block so PE streams without a barrier.
        k_specs = [(tk01, 128, 192), (tk2, 64, 320)]
        q_specs = [(tq01, 128, 0), (tq2, 64, 128)]

        def qk_mm(dst, np_, fo, qb, pool):
            lo = qb * 512
            ps = pool.tile([128, 512], dt.float32, tag="qk", name="qkps")
            for cc in range(NCC):
                nc.tensor.matmul(
                    ps[0:np_, :], wqT[:, cc, fo:fo + np_],
                    xqT[:, cc, lo:lo + 512],
                    start=(cc == 0), stop=(cc == NCC - 1))
            nc.vector.tensor_copy(dst[0:np_, lo:lo + 512], ps[0:np_, :])

        with tc.tile_pool(name="kv_ps", bufs=3, space="PSUM") as kv_ps:
            for dst, np_, fo in k_specs:
                for qb in range(QB):
                    qk_mm(dst, np_, fo, qb, kv_ps)
            for t in range(NT):
                ps = kv_ps.tile([128, 512], dt.float32, tag="qk", name="vps")
                for cc in range(NCC):
                    nc.tensor.matmul(
                        ps[:, 0:CQ], xqT[:, cc, t * 128:(t + 1) * 128],
                        wqT[:, cc, 384:576],
                        start=(cc == 0), stop=(cc == NCC - 1))
                if t < NTF:
                    v_re = vtf[:, t, :].rearrange("p (h x) -> p h x",
                                                  x=D + 1)
                else:
                    v_re = vtb[:, t - NTF, :].rearrange("p (h x) -> p h x",
                                                        x=D + 1)
                nc.vector.tensor_scalar(
                    v_re[:, :, 0:D],
                    ps[:, 0:CQ].rearrange("p (h x) -> p h x", x=D),
                    meanc_q_col[:], None, op0=ALU.mult)

        if debug_taps:
            nc.sync.dma_start(dbg["kt01"].ap(), tk01[:].bitcast(dt.float32))
            nc.sync.dma_start(dbg["vt0"].ap(),
                              vtf[:, 0, :].bitcast(dt.float32))
        stage_es.close()   # release x/wq staging SBUF before attention

        # ---------------- attention + masked pad + ReduceScatter ----------
        # bf16 exchange: halves the collective payload (the cost model bills
        # out-bytes at ~40GB/s); ~0.2% rounding on attention outputs.
        rs_in = dram.tile([N, C], dt.bfloat16)
        rs_out = dram.tile([NQ, C], dt.bfloat16)
        with tc.tile_pool(name="s_ps", bufs=2, space="PSUM") as s_ps, \
             tc.tile_pool(name="av_ps", bufs=3, space="PSUM") as av_ps, \
             tc.tile_pool(name="misc_ps", bufs=1, space="PSUM") as misc_ps, \
             tc.tile_pool(name="aexp", bufs=3) as aexp, \
             tc.tile_pool(name="aexpb", bufs=2) as aexpb, \
             tc.tile_pool(name="avsb", bufs=2) as avsb, \
             tc.tile_pool(name="attq", bufs=2) as attq, \
             tc.tile_pool(name="apad", bufs=2) as apad_p:
            def qk_slices(h):
                if h < 2:
                    return (tk01, 64 * h), (tq01, 64 * h)
                return (tk2, 0), (tq2, 0)

            for qb in range(QB):
                lo = qb * 512
                for dst, np_, fo in q_specs:
                    qk_mm(dst, np_, fo, qb, misc_ps)
                if debug_taps and qb == QB - 1:
                    nc.sync.dma_start(dbg["qt01"].ap(),
                                      tq01[:].bitcast(dt.float32))
                att_qb = attq.tile([128, 4, CQ], dt.float32, tag="attq")
                avs = []
                for h in range(HG):
                    av = av_ps.tile([D + 1, 512], dt.float32, tag="av",
                                    name=f"av{h}")
                    avs.append(av)
                for h in range(HG):
                    (ktt, kpo), (qtt, qpo) = qk_slices(h)
                    av = avs[h]

                    def emit_avs(kvp, ae):
                        for sub in range(2):
                            kv = 2 * kvp + sub
                            vsl = (vtf[:, kv, h * (D + 1):(h + 1) * (D + 1)]
                                   if kv < NTF else
                                   vtb[:, kv - NTF,
                                       h * (D + 1):(h + 1) * (D + 1)])
                            nc.tensor.matmul(
                                av[:], vsl,
                                ae[:, sub * 512:(sub + 1) * 512],
                                start=(kv == 0), stop=(kv == NT - 1))

                    # one-pair software pipeline: AV(i) is emitted after
                    # QK(i+1) so PE never sits behind the exp of pair i.
                    pending = None
                    for kvp in range(NT // 2):
                        sp = s_ps.tile([128, 1024], dt.float32, tag="s",
                                       name="sp")
                        for sub in range(2):
                            kv = 2 * kvp + sub
                            nc.tensor.matmul(
                                sp[:, sub * 512:(sub + 1) * 512],
                                ktt[kpo:kpo + 64,
                                    kv * 128:(kv + 1) * 128],
                                qtt[qpo:qpo + 64, lo:lo + 512],
                                start=True, stop=True)
                        if 2 * kvp < NTF:
                            ae = aexp.tile([128, 1024], dt.float32r,
                                           tag="ae")
                            nc.scalar.activation(ae[:], sp[:], AF.Exp,
                                                 scale=swsq8_col[:])
                        else:
                            ae = aexpb.tile([128, 1024], dt.bfloat16,
                                            tag="aeb")
                            nc.vector.tensor_scalar(
                                ae[:].bitcast(dt.int16), sp[:], EXPA16,
                                EXPB16, op0=ALU.mult, op1=ALU.add)
                        if debug_taps and qb == 0 and h == 0 and kvp == 0:
                            nc.sync.dma_start(dbg["ae0"].ap(),
                                              ae[:].bitcast(dt.float32))
                        if pending is not None:
                            emit_avs(*pending)
                        pending = (kvp, ae)
                    emit_avs(*pending)
                    av_sb = avsb.tile([D + 1, 512], dt.float32, tag="avsb")
                    nc.vector.tensor_copy(av_sb[:], av[:])
                    for tt in range(4):
                        tp = av_ps.tile([128, D + 1], dt.float32, tag="av",
                                        name="tp")
                        nc.tensor.transpose(
                            tp[:, 0:D + 1],
                            av_sb[:, tt * 128:(tt + 1) * 128],
                            ident[0:D + 1, 0:D + 1])
                        dcol = stats.tile([128, 1], dt.float32, tag="dcol")
                        nc.vector.reciprocal(dcol[:], tp[:, D:D + 1])
                        nc.vector.tensor_scalar(
                            att_qb[:, tt, h * D:(h + 1) * D], tp[:, 0:D],
                            dcol[:], None, op0=ALU.mult)
                if debug_taps and qb == 0:
                    nc.sync.dma_start(dbg["att0"].ap(),
                                      att_qb[:].rearrange("p a b -> p (a b)"))
                ap_t = apad_p.tile([128, 4, C], dt.bfloat16, tag="apad")
                for m in range(4):
                    nc.gpsimd.tensor_scalar(
                        ap_t[:, :, m * CQ:(m + 1) * CQ], att_qb[:],
                        mask_bc[:, m:m + 1], None, op0=ALU.mult)
                nc.sync.dma_start(
                    rs_in[lo:lo + 512, :].rearrange(
                        "(s p) c -> p s c", p=128), ap_t[:])
                if qb < 3:
                    # wproj ternarize drip-fed into attention-phase slack
                    for s in (2 * qb, 2 * qb + 1):
                        wp_t = ternarize(wpf[:, s, :], [128, C], thr_p,
                                         nthr_p)
                        nc.scalar.dma_start(
                            wpT[:, :, s * 128:(s + 1) * 128], wp_t[:],
                            transpose=True)
            nc.gpsimd.collective_compute(
                "ReduceScatter", ALU.add,
                replica_groups=[[0, 1, 2, 3], [4, 5, 6, 7]],
                ins=[rs_in.opt()], outs=[rs_out.opt()])

        # ---------------- proj bitlinear -----------------------------------
        with tc.tile_pool(name="recv", bufs=1) as recv_p, \
             tc.tile_pool(name="pscr", bufs=3) as pscr, \
             tc.tile_pool(name="pdump", bufs=2) as pdump, \
             tc.tile_pool(name="xq2T", bufs=1) as xq2T_p, \
             tc.tile_pool(name="m2_ps", bufs=3, space="PSUM") as m2_ps, \
             tc.tile_pool(name="outsb", bufs=2) as outsb:
            recv = recv_p.tile([128, 4, C], dt.bfloat16)
            nc.sync.dma_start(recv[:], rs_out[:].rearrange(
                "(s p) c -> p s c", p=128))
            if debug_taps:
                nc.sync.dma_start(dbg["recv0"].ap(), recv[:, 0, :])
            xq2T = xq2T_p.tile([128, NCC, NQ], dt.bfloat16)
            # fully per-tile so each 128-token slice pipelines quant ->
            # transpose -> matmul -> scale -> DMA without a barrier
            for k in range(4):
                x_t = recv[:, k, :]
                if not g_is_one:
                    xg = pscr.tile([128, C], dt.float32, tag="xg2")
                    nc.vector.tensor_tensor(xg[:], x_t, gp_bc[:],
                                            op=ALU.mult)
                    x_in = xg[:]
                else:
                    x_in = x_t
                sumsq2 = stats.tile([128, 1], dt.float32, tag="ss2")
                xsq = pdump.tile([128, C], dt.float32, tag="xsq2")
                nc.scalar.activation(xsq[:], x_t, AF.Square,
                                     accum_out=sumsq2[:])
                am = stats.tile([128, 1], dt.float32, tag="am")
                nc.vector.tensor_reduce(am[:], x_in, axis=AX.X, op=ALU.max,
                                        apply_absolute_value=True)
                amax2 = stats.tile([128, 1], dt.float32, tag="am2")
                nc.vector.tensor_scalar(amax2[:], am[:], EPS, None,
                                        op0=ALU.max)
                r_am = stats.tile([128, 1], dt.float32, tag="r_am")
                nc.vector.reciprocal(r_am[:], amax2[:])
                s_col = stats.tile([128, 1], dt.float32, tag="s2")
                nc.vector.tensor_scalar(s_col[:], r_am[:], 127.0, None,
                                        op0=ALU.mult)
                t_r = pscr.tile([128, C], dt.float32, tag="t_r2")
                nc.vector.tensor_scalar(t_r[:], x_in, s_col[:], MAGIC,
                                        op0=ALU.mult, op1=ALU.add)
                xq2 = pscr.tile([128, C], dt.bfloat16, tag="xq2")
                nc.vector.tensor_scalar(xq2[:], t_r[:], -MAGIC, None,
                                        op0=ALU.add)
                nc.scalar.dma_start(xq2T[:, :, k * 128:(k + 1) * 128], xq2[:],
                                    transpose=True)
                ms2 = stats.tile([128, 1], dt.float32, tag="ms2")
                nc.vector.tensor_scalar(ms2[:], sumsq2[:], float(1.0 / C),
                                        EPS, op0=ALU.mult, op1=ALU.add)
                rstd2 = _rsqrt_tile(nc, pdump, ms2[:], 1)
                pre2 = stats.tile([128, 1], dt.float32, tag="pre2")
                nc.vector.tensor_tensor(pre2[:], amax2[:], rstd2[:],
                                        op=ALU.mult)
                pcol = stats.tile([128, 1], dt.float32, tag="pcol")
                nc.vector.tensor_scalar(pcol[:], pre2[:], meanc_p_col[:],
                                        None, op0=ALU.mult)
                pcol2 = stats.tile([128, 1], dt.float32, tag="pcol2")
                nc.vector.tensor_scalar(pcol2[:], pcol[:],
                                        float(1.0 / 127.0), None,
                                        op0=ALU.mult)
                o_sb = outsb.tile([128, C], dt.float32, tag="osb")
                for half in range(2):
                    ps = m2_ps.tile([128, 384], dt.float32, tag="m2")
                    for cc in range(NCC):
                        nc.tensor.matmul(
                            ps[:], xq2T[:, cc, k * 128:(k + 1) * 128],
                            wpT[:, cc, half * 384:(half + 1) * 384],
                            start=(cc == 0), stop=(cc == NCC - 1))
                    nc.vector.tensor_scalar(
                        o_sb[:, half * 384:(half + 1) * 384], ps[:],
                        pcol2[:], None, op0=ALU.mult)
                nc.sync.dma_start(out_d.ap()[k * 128:(k + 1) * 128, :],
                                  o_sb[:])

    nc.compile()
    return nc


def _get_program(g_is_one=True, debug_taps=False):
    key = (g_is_one, debug_taps)
    if key not in _CACHE:
        _CACHE[key] = build_program(g_is_one, debug_taps)
    return _CACHE[key]


def kernel(x, w_qkv, g_qkv, w_proj, g_proj, _trace=False, _debug_taps=False,
           **trace_kwargs):
    x = np.ascontiguousarray(np.asarray(x, dtype=np.float32))
    w_qkv = np.ascontiguousarray(np.asarray(w_qkv, dtype=np.float32))
    w_proj = np.ascontiguousarray(np.asarray(w_proj, dtype=np.float32))
    gq = np.ascontiguousarray(np.asarray(g_qkv, dtype=np.float32).reshape(1, C))
    gp = np.ascontiguousarray(np.asarray(g_proj, dtype=np.float32).reshape(1, C))
    g_is_one = bool(np.all(gq == 1.0) and np.all(gp == 1.0))

    nc = _get_program(g_is_one, _debug_taps)
    in_maps = []
    for core in range(8):
        b, g = core // 4, core % 4
        # head-slice rows of w_qkv: Q rows [192g,192g+192), K +768, V +1536
        wqs = np.ascontiguousarray(np.concatenate([
            w_qkv[blk * C + CQ * g: blk * C + CQ * (g + 1)]
            for blk in range(3)], axis=0))
        mask = np.zeros((1, 4), dtype=np.float32)
        mask[0, g] = 1.0
        in_maps.append({
            "xb": x[b],
            "wqs": wqs,
            "wp": w_proj,
            "gq": gq,
            "gp": gp,
            "mask": mask,
        })
    res = run_bass_kernel_spmd(nc, in_maps, list(range(8)), trace=_trace,
                               **trace_kwargs)
    out = np.empty((B, N, C), dtype=np.float32)
    for core in range(8):
        b, g = core // 4, core % 4
        out[b, g * NQ:(g + 1) * NQ] = res.results[core]["out"]
    if _trace or _debug_taps:
        return out, res
    return out
